# revision 1
# baseline (speedup 1.0000x reference)
"""Trainium2 Bass kernel for nn_BranchingLayer (gnn_message_passing).

Computation (reference):
    parents_ftxs = x[idxs_level]                      # identity gather (arange)
    pg           = global_features[parents_idxs % B]  # random gather
    h1 = leaky_relu([parents_ftxs, pg] @ W1 + b1)
    h2 = h1 @ W2 + b2 + repeat(parents_ftxs, 2, -1)
    children = interleave-reshape(h2)                 # [(2p+br)*B + b, f] = h2[p*B+b, br*F+f]
    out = concat([x, children])

Device strategy (8 cores, rows sharded 32768/core = 32 parents):
  - host: gather pg rows and pre-transpose to [64, rows] per core
  - per 512-row group: DMA x rows + pgT cols; PE-transpose x tiles; fp32r matmuls
    (mm1 -> h1^T in PSUM, leaky-relu on ACT -> SBUF, mm2 row-major with W2 streamed
     and h1^T/x^T as stationary; residual folded in as a third matmul against a
     0/1 repeat matrix); DVE adds b2 while splitting branch channels; DMA children out.
  - host: concat [x, children].
"""

import sys

import numpy as np

try:
    import ml_dtypes
except ImportError:
    ml_dtypes = None

if "/opt/trn_rl_repo" not in sys.path:
    sys.path.insert(0, "/opt/trn_rl_repo")

N_PARENTS = 256
BATCH = 1024
N_FEAT = 128
N_BR = 2
N_GLOBAL = 64
N_CORES = 8
ROWS = N_PARENTS * BATCH            # 262144
RPC = ROWS // N_CORES               # 32768 rows per core
CPC = RPC * N_BR                    # 65536 child rows per core
GROUP = 512                         # rows per pipeline group
N_GROUPS = RPC // GROUP             # 64
HID = 256

# leaky-relu implementation: "lrelu" (single ACT op, HW Lrelu table),
# "relu2" (Identity + Relu + DVE add, guaranteed semantics)
LRELU_MODE = "lrelu"

_CACHE = {}


def _split_multiwait(nc, mybir):
    """This image's walrus accepts only one sync-wait per instruction; hoist
    extra waits onto same-engine NOPs inserted before the instruction."""
    for f in nc.m.functions:
        for bb in f.blocks:
            new_insts = []
            changed = False
            for inst in bb.instructions:
                si = inst.sync_info
                if si is not None and len(si.on_wait) > 1:
                    waits = list(si.on_wait)
                    for w in waits[:-1]:
                        new_insts.append(
                            mybir.InstNoOp(
                                name=nc.get_next_instruction_name(),
                                engine=inst.engine,
                                sync_info=mybir.SyncInfo(on_wait=[w], on_update=[]),
                            )
                        )
                    inst.sync_info = mybir.SyncInfo(
                        on_wait=[waits[-1]], on_update=list(si.on_update)
                    )
                    changed = True
                new_insts.append(inst)
            if changed:
                bb.instructions = new_insts


def _build_program(lrelu_mode=LRELU_MODE, split_waits=True):
    key = ("prog", lrelu_mode, split_waits)
    if key in _CACHE:
        return _CACHE[key]

    import concourse.bass as bass
    import concourse.mybir as mybir
    import concourse.tile as tile

    f32 = mybir.dt.float32
    f32r = mybir.dt.float32r
    bf16 = mybir.dt.bfloat16

    def r(ap):
        return ap.bitcast(f32r)

    nc = bass.Bass()
    xs = nc.declare_dram_parameter("xs", [RPC, N_FEAT], f32, isOutput=False)
    pgt = nc.declare_dram_parameter("pgt", [N_GLOBAL, RPC], bf16, isOutput=False)
    w1 = nc.declare_dram_parameter("w1", [N_FEAT + N_GLOBAL, HID], f32r, isOutput=False)
    w1bh = nc.declare_dram_parameter("w1bh", [N_GLOBAL, HID], bf16, isOutput=False)
    b1c = nc.declare_dram_parameter("b1c", [128, 2], f32, isOutput=False)
    b1n = nc.declare_dram_parameter("b1n", [128, 2], f32, isOutput=False)
    w2 = nc.declare_dram_parameter("w2", [HID, HID], f32r, isOutput=False)
    b2t = nc.declare_dram_parameter("b2t", [128, 2 * HID], f32, isOutput=False)
    rmat = nc.declare_dram_parameter("rmat", [N_FEAT, HID], f32r, isOutput=False)
    ident = nc.declare_dram_parameter("ident", [128, 128], f32, isOutput=False)
    ch = nc.declare_dram_parameter("ch", [CPC, N_FEAT], f32, isOutput=True)

    AF = mybir.ActivationFunctionType

    with tile.TileContext(nc) as tc:
        with (
            tc.tile_pool(name="const", bufs=1) as cpool,
            tc.tile_pool(name="xin", bufs=4) as xpool,
            tc.tile_pool(name="pg", bufs=4) as gpool,
            tc.tile_pool(name="xt", bufs=3) as xtpool,
            tc.tile_pool(name="h1", bufs=3) as h1pool,
            tc.tile_pool(name="tmp", bufs=2) as tpool,
            tc.tile_pool(name="cout", bufs=4) as opool,
            tc.tile_pool(name="psA", bufs=1, space="PSUM") as psA,
            tc.tile_pool(name="psB", bufs=4, space="PSUM") as psB,
            tc.tile_pool(name="psC", bufs=3, space="PSUM") as psC,
        ):
            w1a = cpool.tile([128, HID], f32r)
            nc.sync.dma_start(w1a[:], w1[0:128, :])
            w1b = cpool.tile([64, HID], bf16)
            nc.sync.dma_start(w1b[:], w1bh[:, :])
            w2a = cpool.tile([128, HID], f32r)
            nc.sync.dma_start(w2a[:], w2[0:128, :])
            w2b = cpool.tile([128, HID], f32r)
            nc.sync.dma_start(w2b[:], w2[128:256, :])
            rm = cpool.tile([128, HID], f32r)
            nc.sync.dma_start(rm[:], rmat[:])
            b2s = cpool.tile([128, 2 * HID], f32)
            nc.sync.dma_start(b2s[:], b2t[:])
            b1s = cpool.tile([128, 2], f32)
            nc.sync.dma_start(b1s[:], b1c[:])
            b1ns = cpool.tile([128, 2], f32)
            nc.sync.dma_start(b1ns[:], b1n[:])
            idn = cpool.tile([128, 128], f32)
            nc.sync.dma_start(idn[:], ident[:])

            b2s4 = b2s[:, :].rearrange("p (t b k) -> p b t k", t=2, b=2)

            def emit_lrelu(st):
                h1 = [h1pool.tile([128, GROUP], f32r, tag="h1sb", name=f"h1sb{m_}")
                      for m_ in range(2)]
                for hh in range(2):
                    for m in range(2):
                        if lrelu_mode != "lrelu":
                            continue
                        hs = slice(hh * 256, (hh + 1) * 256)
                        nc.scalar.activation(
                            h1[m][:, hs], st["h1ps"][m][:, hs], AF.Lrelu,
                            bias=b1s[:, m:m + 1], scale=1.0, alpha=0.01,
                        )
                for m in range(2):
                    if lrelu_mode == "lrelu":
                        pass
                    else:
                        # leaky(u) = u + 0.99*relu(-u), u = x + b1
                        v = tpool.tile([128, GROUP], f32, tag="v")
                        nc.scalar.activation(
                            h1[m][:, :], st["h1ps"][m][:, :], AF.Identity,
                            bias=b1s[:, m:m + 1],
                        )
                        nc.scalar.activation(
                            v[:, :], st["h1ps"][m][:, :], AF.Relu,
                            bias=b1ns[:, m:m + 1], scale=-0.99,
                        )
                        nc.vector.tensor_add(h1[m][:, :], h1[m][:, :], v[:, :])
                st["h1"] = h1

            def emit_tail(st):
                # DVE: add b2 while splitting branch channels; then store children
                cht = opool.tile([128, 2 * GROUP], f32)
                cht4 = cht[:, :].rearrange("p (b t k) -> p b t k", b=2, t=4)
                for t2 in range(2):
                    nc.vector.tensor_add(
                        cht4[:, :, t2::2, :],
                        st["h2ps"][t2][:, :].rearrange("p (t b k) -> p b t k", t=2, b=2),
                        b2s4[:, :, :, :],
                    )
                g = st["g"]
                p_local, half = divmod(g, 2)
                base0 = (2 * p_local) * BATCH + half * GROUP
                base1 = (2 * p_local + 1) * BATCH + half * GROUP
                nc.sync.dma_start(
                    ch[base0:base0 + GROUP, :].rearrange("(p i) f -> p (i f)", i=4),
                    cht[:, 0:GROUP],
                )
                nc.sync.dma_start(
                    ch[base1:base1 + GROUP, :].rearrange("(p i) f -> p (i f)", i=4),
                    cht[:, GROUP:2 * GROUP],
                )

            prev = None
            for g in range(N_GROUPS + 1):
                cur = None
                mm1_thunks = []
                if g < N_GROUPS:
                    # ---- loads: partition p holds rows 4p..4p+3 (2KB descs) ----
                    xg = xpool.tile([128, GROUP], f32)
                    nc.sync.dma_start(
                        xg[:, :],
                        xs[g * GROUP:(g + 1) * GROUP, :]
                        .rearrange("(p i) f -> p (i f)", i=4),
                    )
                    pgg = gpool.tile([64, GROUP], bf16)
                    nc.sync.dma_start(pgg[:, :], pgt[:, g * GROUP:(g + 1) * GROUP])

                    # ---- transpose "subtiles" (row sets {4p+s}) ----
                    xt_ps = psA.tile([128, GROUP], f32)
                    for s in range(4):
                        nc.tensor.transpose(
                            xt_ps[:, s * 128:(s + 1) * 128],
                            xg[:, s * 128:(s + 1) * 128],
                            idn[:, :],
                        )
                    xt = xtpool.tile([128, GROUP], f32r)
                    nc.scalar.copy(xt[:, :], xt_ps[:, :])

                    # ---- mm1 thunks (interleaved into prev group's mm2 stream) ----
                    h1ps = [psB.tile([128, GROUP], f32, tag="h1ps", name=f"h1ps{m_}")
                            for m_ in range(2)]
                    cur = {"g": g, "xt": xt, "h1ps": h1ps}
                    for m_ in range(2):
                        mm1_thunks.append(lambda m=m_: nc.tensor.matmul(
                            h1ps[m][:, :], w1a[:, m * 128:(m + 1) * 128], xt[:, :],
                            start=True, stop=False,
                        ))
                    for m_ in range(2):
                        mm1_thunks.append(lambda m=m_: nc.tensor.matmul(
                            h1ps[m][:, :], w1b[:, m * 128:(m + 1) * 128], pgg[:, :],
                            start=False, stop=True,
                        ))

                if prev is not None:
                    # ---- mm2(prev) with mm1(cur) MMs woven into the stream ----
                    h2ps = [psC.tile([128, 2 * HID], f32, tag="h2ps", name=f"h2ps{t_}")
                            for t_ in range(2)]
                    prev["h2ps"] = h2ps
                    h1p, xtp = prev["h1"], prev["xt"]
                    steps = ((h1p[0], w2a), (h1p[1], w2b), (xtp, rm))
                    mm2_thunks = []
                    for j in range(2):
                        for step, (lh, rh) in enumerate(steps):
                            for t2 in range(2):
                                s = t2 + 2 * j
                                mm2_thunks.append(
                                    lambda t2=t2, j=j, lh=lh, rh=rh, step=step,
                                    ssl=slice(s * 128, (s + 1) * 128):
                                    nc.tensor.matmul(
                                        h2ps[t2][:, j * HID:(j + 1) * HID],
                                        lh[:, ssl], rh[:, :],
                                        start=(step == 0), stop=(step == 2),
                                    ))
                    # weave: 3 mm2 MMs, then 1 mm1 MM, ...
                    for th in mm2_thunks:
                        th()
                    for th in mm1_thunks:
                        th()
                    if cur is not None:
                        emit_lrelu(cur)
                    emit_tail(prev)
                else:
                    for th in mm1_thunks:
                        th()
                    emit_lrelu(cur)

                prev = cur

    if split_waits:
        _split_multiwait(nc, mybir)
    _CACHE[key] = nc
    return nc


def _host_prep(x, global_features, W1, b1, W2, b2, idxs_level, parents_idxs):
    x = np.ascontiguousarray(np.asarray(x, dtype=np.float32))
    G = np.asarray(global_features, dtype=np.float32)
    W1 = np.ascontiguousarray(np.asarray(W1, dtype=np.float32))
    b1 = np.asarray(b1, dtype=np.float32)
    W2 = np.ascontiguousarray(np.asarray(W2, dtype=np.float32))
    b2 = np.asarray(b2, dtype=np.float32)
    idxs = np.asarray(idxs_level)
    pidx = np.asarray(parents_idxs)

    if np.array_equal(idxs, np.arange(ROWS, dtype=idxs.dtype)):
        xg = x
    else:  # general gather fallback (host)
        xg = np.ascontiguousarray(x[idxs])

    pg = G[pidx % BATCH]                              # [ROWS, 64]
    # device row permutation: within each 512-row group, SBUF partition p holds
    # rows 4p..4p+3 and "subtile" s is the row set {4p+s}; pgT columns follow
    # (s, p) order, i.e. column s*128+p corresponds to row 4p+s.
    pgt = np.ascontiguousarray(
        pg.reshape(N_CORES, N_GROUPS, 128, 4, N_GLOBAL).transpose(0, 4, 1, 3, 2)
        .reshape(N_CORES, N_GLOBAL, RPC)
    ).astype(ml_dtypes.bfloat16)                      # [8, 64, RPC]
    w1bh = W1[128:192, :].astype(ml_dtypes.bfloat16)

    b1c = np.ascontiguousarray(b1.reshape(2, 128).T)  # [128, 2]
    b1n = np.ascontiguousarray((-0.99 * b1).reshape(2, 128).T)
    b2t = np.ascontiguousarray(np.broadcast_to(np.tile(b2, 2), (128, 2 * HID)))
    rmat = np.zeros((N_FEAT, HID), dtype=np.float32)
    k = np.arange(N_FEAT)
    rmat[k, 2 * k] = 1.0
    rmat[k, 2 * k + 1] = 1.0
    ident = np.eye(128, dtype=np.float32)

    in_maps = []
    for c in range(N_CORES):
        in_maps.append({
            "xs": xg[c * RPC:(c + 1) * RPC],
            "pgt": pgt[c],
            "w1": W1,
            "w1bh": w1bh,
            "b1c": b1c,
            "b1n": b1n,
            "w2": W2,
            "b2t": b2t,
            "rmat": rmat,
            "ident": ident,
        })
    return x, in_maps


def kernel(x, global_features, W1, b1, W2, b2, idxs_level, parents_idxs,
           _trace=False, _trace_kwargs=None):
    from concourse.bass_utils import run_bass_kernel_spmd

    x_np, in_maps = _host_prep(
        x, global_features, W1, b1, W2, b2, idxs_level, parents_idxs
    )
    nc = _build_program()
    res = run_bass_kernel_spmd(
        nc, in_maps, list(range(N_CORES)),
        trace=_trace, **(_trace_kwargs or {}),
    )
    children = np.concatenate(
        [res.results[c]["ch"] for c in range(N_CORES)], axis=0
    )
    out = np.concatenate([x_np, children], axis=0)
    if _trace:
        kernel.last_result = res
    return out



# revision 3
# speedup vs baseline: 1.3306x; 1.3306x over previous
"""Trainium2 Bass kernel for nn_BranchingLayer (gnn_message_passing).

Computation (reference):
    parents_ftxs = x[idxs_level]                      # identity gather (arange)
    pg           = global_features[parents_idxs % B]  # random gather
    h1 = leaky_relu([parents_ftxs, pg] @ W1 + b1)
    h2 = h1 @ W2 + b2 + repeat(parents_ftxs, 2, -1)
    children = interleave-reshape(h2)                 # [(2p+br)*B + b, f] = h2[p*B+b, br*F+f]
    out = concat([x, children])

Device strategy (8 cores, rows sharded 32768/core = 32 parents), v2:
  All compute stays in the transposed (feature-major) domain so every matmul
  has weights stationary and rows moving (N=512), all in bf16:
    - host: xT [128, rows], pgT duplicated into both partition halves
      [128, rows] (bf16), per core.
    - mm1: h1T[m] = W1x_m^T xT (+) W1g_m^T pgT, the two K=64 global-feature
      matmuls packed into one concurrent row-tile pair (tile_position 0/64).
    - ACT: leaky-relu + b1 (per-partition bias in this domain) -> h1 bf16.
    - mm2: h2T[oh] = W2 blocks^T h1 (+) residual via a 0/1 repeat matrix,
      packed as a second K=64 row-tile pair streaming xT halves.
    - DVE: + b2 (per-partition here) while casting PSUM -> bf16 SBUF.
    - out DMA: chT [2, 128, rows] bf16, one DMA per 4-group supergroup.
  Host untangles chT into child-row order and concats [x, children].
"""

import sys

import numpy as np

try:
    import ml_dtypes
except ImportError:
    ml_dtypes = None

if "/opt/trn_rl_repo" not in sys.path:
    sys.path.insert(0, "/opt/trn_rl_repo")

N_PARENTS = 256
BATCH = 1024
N_FEAT = 128
N_BR = 2
N_GLOBAL = 64
N_CORES = 8
ROWS = N_PARENTS * BATCH            # 262144
RPC = ROWS // N_CORES               # 32768 rows per core
PPC = N_PARENTS // N_CORES          # 32 parents per core
CPC = RPC * N_BR                    # 65536 child rows per core
GROUP = 512                         # rows per pipeline group
SGROUP = 4                          # groups per DMA supergroup
N_SG = RPC // (GROUP * SGROUP)      # 16
HID = 256

_CACHE = {}


def _split_multiwait(nc, mybir):
    """This image's walrus accepts only one sync-wait per instruction; hoist
    extra waits onto same-engine NOPs inserted before the instruction."""
    for f in nc.m.functions:
        for bb in f.blocks:
            new_insts = []
            changed = False
            for inst in bb.instructions:
                si = inst.sync_info
                if si is not None and len(si.on_wait) > 1:
                    waits = list(si.on_wait)
                    for w in waits[:-1]:
                        new_insts.append(
                            mybir.InstNoOp(
                                name=nc.get_next_instruction_name(),
                                engine=inst.engine,
                                sync_info=mybir.SyncInfo(on_wait=[w], on_update=[]),
                            )
                        )
                    inst.sync_info = mybir.SyncInfo(
                        on_wait=[waits[-1]], on_update=list(si.on_update)
                    )
                    changed = True
                new_insts.append(inst)
            if changed:
                bb.instructions = new_insts


def _build_program():
    key = ("prog_v2",)
    if key in _CACHE:
        return _CACHE[key]

    import concourse.bass as bass
    import concourse.mybir as mybir
    import concourse.tile as tile

    f32 = mybir.dt.float32
    bf16 = mybir.dt.bfloat16

    nc = bass.Bass()
    xt = nc.declare_dram_parameter("xt", [N_FEAT, RPC], bf16, isOutput=False)
    pgd = nc.declare_dram_parameter("pgd", [128, RPC], bf16, isOutput=False)
    w1x = nc.declare_dram_parameter("w1x", [N_FEAT, HID], bf16, isOutput=False)
    w1g = nc.declare_dram_parameter("w1g", [128, 128], bf16, isOutput=False)
    w2d = nc.declare_dram_parameter("w2d", [HID, HID], bf16, isOutput=False)
    rcd = nc.declare_dram_parameter("rcd", [128, 128], bf16, isOutput=False)
    b1d = nc.declare_dram_parameter("b1d", [128, 2], f32, isOutput=False)
    b2d = nc.declare_dram_parameter("b2d", [128, 2], f32, isOutput=False)
    cht = nc.declare_dram_parameter("cht", [2, 128, RPC], bf16, isOutput=True)

    AF = mybir.ActivationFunctionType

    with tile.TileContext(nc) as tc:
        with (
            tc.tile_pool(name="const", bufs=1) as cpool,
            tc.tile_pool(name="xin", bufs=2) as xpool,
            tc.tile_pool(name="gin", bufs=2) as gpool,
            tc.tile_pool(name="h1", bufs=3) as h1pool,
            tc.tile_pool(name="cout", bufs=2) as opool,
            tc.tile_pool(name="ps1", bufs=4, space="PSUM") as ps1,
            tc.tile_pool(name="ps2", bufs=4, space="PSUM") as ps2,
        ):
            w1xs = cpool.tile([128, HID], bf16)
            nc.sync.dma_start(w1xs[:], w1x[:, :])
            w1gs = cpool.tile([128, 128], bf16)
            nc.sync.dma_start(w1gs[:], w1g[:, :])
            w2a = cpool.tile([128, HID], bf16)
            nc.sync.dma_start(w2a[:], w2d[0:128, :])
            w2b = cpool.tile([128, HID], bf16)
            nc.sync.dma_start(w2b[:], w2d[128:256, :])
            rcs = cpool.tile([128, 128], bf16)
            nc.sync.dma_start(rcs[:], rcd[:])
            b1s = cpool.tile([128, 2], f32)
            nc.sync.dma_start(b1s[:], b1d[:])
            b2s = cpool.tile([128, 2], f32)
            nc.sync.dma_start(b2s[:], b2d[:])

            for sg in range(N_SG):
                c0 = sg * SGROUP * GROUP
                c1 = c0 + SGROUP * GROUP
                xg = xpool.tile([128, SGROUP * GROUP], bf16, tag="xg")
                nc.sync.dma_start(xg[:, :], xt[:, c0:c1])
                gg = gpool.tile([128, SGROUP * GROUP], bf16, tag="gg")
                nc.sync.dma_start(gg[:, :], pgd[:, c0:c1])
                og = opool.tile([128, 2 * SGROUP * GROUP], bf16, tag="og")
                og3 = og[:, :].rearrange("p (oh c) -> p oh c", oh=2)

                for gl in range(SGROUP):
                    cs = slice(gl * GROUP, (gl + 1) * GROUP)
                    xs_ = xg[:, cs]
                    gs_ = gg[:, cs]

                    # ---- mm1: h1T[m] = W1x_m^T @ xT + W1g_m^T @ pgT ----
                    h1ps = [ps1.tile([128, GROUP], f32, tag="h1ps",
                                     name=f"h1ps{m_}") for m_ in range(2)]
                    for m in range(2):
                        nc.tensor.matmul(
                            h1ps[m][:, :], w1xs[:, m * 128:(m + 1) * 128], xs_,
                            start=True, stop=False,
                        )
                    nc.tensor.matmul(
                        h1ps[0][:, :], w1gs[0:64, :], gs_[0:64, :],
                        start=False, stop=True, tile_position=(0, 0),
                    )
                    nc.tensor.matmul(
                        h1ps[1][:, :], w1gs[64:128, :], gs_[64:128, :],
                        start=False, stop=True, tile_position=(64, 0),
                    )

                    # ---- leaky-relu + b1 (per-partition bias), cast bf16 ----
                    h1 = [h1pool.tile([128, GROUP], bf16, tag="h1sb",
                                      name=f"h1sb{m_}") for m_ in range(2)]
                    for m in range(2):
                        nc.scalar.activation(
                            h1[m][:, :], h1ps[m][:, :], AF.Lrelu,
                            bias=b1s[:, m:m + 1], scale=1.0, alpha=0.01,
                        )

                    # ---- mm2: h2T[oh] = W2^T @ h1 + R^T @ xT (residual) ----
                    h2ps = [ps2.tile([128, GROUP], f32, tag="h2ps",
                                     name=f"h2ps{o_}") for o_ in range(2)]
                    for oh in range(2):
                        nc.tensor.matmul(
                            h2ps[oh][:, :], w2a[:, oh * 128:(oh + 1) * 128],
                            h1[0][:, :], start=True, stop=False,
                        )
                        nc.tensor.matmul(
                            h2ps[oh][:, :], w2b[:, oh * 128:(oh + 1) * 128],
                            h1[1][:, :], start=False, stop=False,
                        )
                    nc.tensor.matmul(
                        h2ps[0][:, :], rcs[0:64, :], xs_[0:64, :],
                        start=False, stop=True, tile_position=(0, 0),
                    )
                    nc.tensor.matmul(
                        h2ps[1][:, :], rcs[64:128, :], xs_[64:128, :],
                        start=False, stop=True, tile_position=(64, 0),
                    )

                    # ---- + b2 (per-partition here) while casting to bf16 ----
                    for oh in range(2):
                        nc.vector.tensor_scalar_add(
                            og3[:, oh, cs], h2ps[oh][:, :], b2s[:, oh:oh + 1],
                        )

                nc.sync.dma_start(
                    cht[:, :, c0:c1].rearrange("oh p c -> p oh c"),
                    og3[:, :, :],
                )

    import concourse.mybir as mybir
    _split_multiwait(nc, mybir)
    _CACHE[key] = nc
    return nc


def _host_prep(x, global_features, W1, b1, W2, b2, idxs_level, parents_idxs):
    bf = ml_dtypes.bfloat16
    x = np.ascontiguousarray(np.asarray(x, dtype=np.float32))
    G = np.asarray(global_features, dtype=np.float32)
    W1 = np.asarray(W1, dtype=np.float32)
    b1 = np.asarray(b1, dtype=np.float32)
    W2 = np.asarray(W2, dtype=np.float32)
    b2 = np.asarray(b2, dtype=np.float32)
    idxs = np.asarray(idxs_level)
    pidx = np.asarray(parents_idxs)

    if np.array_equal(idxs, np.arange(ROWS, dtype=idxs.dtype)):
        xg = x
    else:  # general gather fallback (host)
        xg = np.ascontiguousarray(x[idxs])

    # per-core transposed x: [8, 128, RPC]
    xt = np.ascontiguousarray(
        xg.reshape(N_CORES, RPC, N_FEAT).transpose(0, 2, 1)
    ).astype(bf)
    # per-core transposed gathered globals, duplicated into both halves
    pg = G[pidx % BATCH]                                  # [ROWS, 64]
    pgt = pg.reshape(N_CORES, RPC, N_GLOBAL).transpose(0, 2, 1).astype(bf)
    pgd = np.ascontiguousarray(np.concatenate([pgt, pgt], axis=1))  # [8,128,RPC]

    w1x = W1[0:128, :].astype(bf)                          # [128, 256]
    w1g = np.ascontiguousarray(
        np.concatenate([W1[128:192, 0:128], W1[128:192, 128:256]], axis=0)
    ).astype(bf)                                           # [128, 128]
    w2d = W2.astype(bf)                                    # [256, 256]
    rp = np.zeros((64, 128), dtype=np.float32)
    rp[np.arange(128) // 2, np.arange(128)] = 1.0
    rcd = np.ascontiguousarray(np.concatenate([rp, rp], axis=0)).astype(bf)
    b1d = np.ascontiguousarray(b1.reshape(2, 128).T)       # [128, 2]
    b2d = np.ascontiguousarray(b2.reshape(2, 128).T)       # [128, 2]

    in_maps = []
    for c in range(N_CORES):
        in_maps.append({
            "xt": xt[c],
            "pgd": pgd[c],
            "w1x": w1x,
            "w1g": w1g,
            "w2d": w2d,
            "rcd": rcd,
            "b1d": b1d,
            "b2d": b2d,
        })
    return x, in_maps


def kernel(x, global_features, W1, b1, W2, b2, idxs_level, parents_idxs,
           _trace=False, _trace_kwargs=None):
    from concourse.bass_utils import run_bass_kernel_spmd

    x_np, in_maps = _host_prep(
        x, global_features, W1, b1, W2, b2, idxs_level, parents_idxs
    )
    nc = _build_program()
    res = run_bass_kernel_spmd(
        nc, in_maps, list(range(N_CORES)),
        trace=_trace, **(_trace_kwargs or {}),
    )
    # cht[core][oh, f, pl*1024 + b] -> children[(core*32 + pl)*2 + oh, b, f]
    parts = []
    for c in range(N_CORES):
        a = np.asarray(res.results[c]["cht"])              # [2, 128, RPC] bf16
        a = a.reshape(2, 128, PPC, BATCH).transpose(2, 0, 3, 1)
        parts.append(a.reshape(CPC, N_FEAT).astype(np.float32))
    children = np.concatenate(parts, axis=0)
    out = np.concatenate([x_np, children], axis=0)
    if _trace:
        kernel.last_result = res
    return out
